# revision 9
# baseline (speedup 1.0000x reference)
"""Eval-mode ClassConditionalBatchNorm2d on 8 Trainium2 NeuronCores.

Math: for each sample b with label l:
    use_class = (alpha > 0) & (class_counts[l] >= 100)
    mean/var  = blend of (global, class[l]) stats if use_class else global
    out       = (x - mean) / sqrt(var + eps) * weight + bias

This folds to a per-(sample, channel) affine:  out = x * scale + shift with
    scale[b,c] = weight[c] / sqrt(var[b,c] + eps)
    shift[b,c] = bias[c] - mean[b,c] * scale[b,c]

The [B=64, C=256] scale/shift tables are tiny (64 KB) and computed on host;
the device kernel streams x (196 MiB) through SBUF applying one fused DVE
tensor_scalar (mult+add, per-partition scalars) per tile — memory-bound.

Sharding: pure data parallel over batch. Each of the 8 cores gets 8 samples
(x shard [8, 256, 56*56]) plus its own [128, 32] scale/shift table arranged
so that column 4*b + 2*h + {0,1} holds (scale, shift) for sample b, channel
half h, with channels on partitions.
"""

import numpy as np
from contextlib import ExitStack

B, C, H, W = 64, 256, 56, 56
HW = H * W
N_CORES = 8
BPC = B // N_CORES  # samples per core
N_HALF = C // 128   # channel halves (partition tiles)
EPS = 1e-5
MIN_COUNT = 100.0

_PROGRAM_CACHE = {}
LAST_RESULTS = None  # BassKernelResults of the most recent run (for profiling)


def _build_program(iters=1, bufs=6, dyn_loop=None, in_place=False):
    """Build + compile the single-core SPMD Bass program (cached).

    iters > 1 repeats the identical sweep back-to-back inside one NEFF;
    dyn_loop=N wraps the sweep in a hardware For loop of N trips. Both are
    used only by the benchmark harness to measure per-sweep cost.
    in_place applies the affine into the input tile (one pool, more bufs).
    """
    key = (iters, bufs, dyn_loop, in_place)
    if key in _PROGRAM_CACHE:
        return _PROGRAM_CACHE[key]

    import concourse.tile as tile
    from concourse import bacc, mybir

    f32 = mybir.dt.float32
    nc = bacc.Bacc(
        "TRN2", target_bir_lowering=False, debug=False, num_devices=N_CORES
    )
    x_ap = nc.dram_tensor("x", [BPC, C, HW], f32, kind="ExternalInput").ap()
    tab_ap = nc.dram_tensor(
        "tables", [128, BPC * N_HALF * 2], f32, kind="ExternalInput"
    ).ap()
    out_ap = nc.dram_tensor("out", [BPC, C, HW], f32, kind="ExternalOutput").ap()

    with tile.TileContext(nc) as tc:
        with ExitStack() as ctx:
            tabp = ctx.enter_context(tc.tile_pool(name="tab", bufs=1))
            xp = ctx.enter_context(tc.tile_pool(name="xs", bufs=bufs))
            outp = ctx.enter_context(tc.tile_pool(name="os", bufs=bufs))

            tab = tabp.tile([128, BPC * N_HALF * 2], f32)
            nc.sync.dma_start(tab[:], tab_ap[:])

            def sweep():
                for b in range(BPC):
                    for h in range(N_HALF):
                        r = N_HALF * b + h
                        t = xp.tile([128, HW], f32)
                        nc.sync.dma_start(t[:], x_ap[b, 128 * h : 128 * (h + 1), :])
                        o = t if in_place else outp.tile([128, HW], f32)
                        nc.vector.tensor_scalar(
                            o[:],
                            t[:],
                            tab[:, 2 * r : 2 * r + 1],
                            tab[:, 2 * r + 1 : 2 * r + 2],
                            mybir.AluOpType.mult,
                            mybir.AluOpType.add,
                        )
                        nc.sync.dma_start(out_ap[b, 128 * h : 128 * (h + 1), :], o[:])

            if dyn_loop is not None:
                with tc.For_i(0, dyn_loop, 1):
                    for _ in range(iters):
                        sweep()
            else:
                for _ in range(iters):
                    sweep()

    nc.compile()
    _PROGRAM_CACHE[key] = nc
    return nc


def _scale_shift(labels, weight, bias, global_mean, global_var,
                 class_mean, class_var, class_counts, alpha):
    """Per-sample affine tables [B, C], mirroring the reference's f32 branch
    selection exactly; the weight/sqrt fold is done in f64 for accuracy."""
    labels = np.asarray(labels).astype(np.int64).reshape(-1)
    a = np.float32(np.asarray(alpha).reshape(()))
    one_m_a = np.float32(1.0) - a

    use_class = (float(a) > 0.0) & (
        np.asarray(class_counts, np.float32)[labels] >= np.float32(MIN_COUNT)
    )  # [B]
    gm = np.asarray(global_mean, np.float32)
    gv = np.asarray(global_var, np.float32)
    blend_mean = one_m_a * gm[None, :] + a * np.asarray(class_mean, np.float32)[labels]
    blend_var = np.clip(
        one_m_a * gv[None, :] + a * np.asarray(class_var, np.float32)[labels],
        np.float32(EPS),
        None,
    )
    mean = np.where(use_class[:, None], blend_mean, gm[None, :])  # [B, C] f32
    var = np.where(use_class[:, None], blend_var, gv[None, :])

    scale64 = np.asarray(weight, np.float64)[None, :] / np.sqrt(
        var.astype(np.float64) + np.float64(EPS)
    )
    shift64 = np.asarray(bias, np.float64)[None, :] - mean.astype(np.float64) * scale64
    return scale64.astype(np.float32), shift64.astype(np.float32)


def kernel(x, labels, weight, bias, global_mean, global_var,
           class_mean, class_var, class_counts, alpha):
    global LAST_RESULTS
    from concourse.bass_utils import run_bass_kernel_spmd

    x = np.asarray(x, np.float32)
    scale, shift = _scale_shift(
        labels, weight, bias, global_mean, global_var,
        class_mean, class_var, class_counts, alpha,
    )

    nc = _build_program()

    in_maps = []
    for c in range(N_CORES):
        xs = x[c * BPC : (c + 1) * BPC].reshape(BPC, C, HW)
        sc = scale[c * BPC : (c + 1) * BPC].reshape(BPC, N_HALF, 128)
        sh = shift[c * BPC : (c + 1) * BPC].reshape(BPC, N_HALF, 128)
        st = np.stack([sc, sh], axis=-1)  # [b, h, p, 2]
        tab = np.ascontiguousarray(
            st.transpose(2, 0, 1, 3).reshape(128, BPC * N_HALF * 2)
        )  # col = 4b + 2h + k
        in_maps.append({"x": np.ascontiguousarray(xs), "tables": tab})

    res = run_bass_kernel_spmd(nc, in_maps, list(range(N_CORES)))
    LAST_RESULTS = res

    out = np.empty((B, C, H, W), np.float32)
    for c in range(N_CORES):
        out[c * BPC : (c + 1) * BPC] = res.results[c]["out"].reshape(BPC, C, H, W)
    return out
